# revision 15
# baseline (speedup 1.0000x reference)
"""ArcFace loss (B=1024, D=512, C=50000) distributed over 8 TRN2 NeuronCores.

Classification/tensor parallel (per the sharding hint): weight rows are
split 6250/core (padded to 6272 = 49 class tiles of 128).

Host staging (layout/dtype only):
  - wt8  [128, KC, CP] fp8_e4m3: transposed lhsT image of the shard,
    scaled by 256 (pad classes 0 -> logit 0, exp 1, negligible vs S~3e5),
    partition-major so the shard loads in a few contiguous DMAs and
    stays resident in SBUF (25KB/partition).
  - wrow [CP, D] bf16: row image for the norm pass + target gather
    (pad rows 1.0 -> finite rsqrt).

Device:
  - prologue DMAs ordered for overlap: embeddings first and weights on
    the sync HWDGE ring, bf16 rows via SWDGE on the gpsimd ring.
  - embeddings normalized on device (ScalarE Square+accum, DVE
    Newton-rsqrt), scaled by 64, bf16 (ScalarE Copy), PE-transposed,
    ScalarE-cast to fp8 enT [128, KC, B].
  - main GEMM: fp8 DoubleRow matmuls (2 k-chunks/instr, 0.5 cyc/row)
    accumulate raw dots in FP32 PSUM; activation scale
    64/(256*64*||w||) recovers 64*cos (classes on partitions).
  - exp: mostly ScalarE table exp over one [128,1024] PSUM read/tile,
    a few tiles via DVE Schraudolph (i16 = A*s*psum + B, bitcast bf16);
    DVE accumulates bf16 exp tiles into two alternating accumulators.
  - per-class norms: fused square+row-sum over bf16 rows, split between
    ScalarE (Square+accum) and DVE (scalar_tensor_tensor), with a slim
    batched Newton-rsqrt per 7-tile chunk issued one chunk ahead.
  - target-class path early: indirect-gather of w[label] bf16 rows,
    cos_t/phi and exp corrections in f32.
  - partial sums reduced over class partitions via PE transpose + DVE
    row reduction; one 12KB AllReduce {S, dd, pp}; every core finishes
    ln Z - 64*phi, mean; core 0's scalar is returned.
"""

import numpy as np
import ml_dtypes

try:
    import concourse.bass as bass  # noqa: F401
except ImportError:  # fallback when PYTHONPATH lacks the repo
    import sys

    for p in ("/opt/trn_rl_repo", "/root/.axon_site/_ro/trn_rl_repo"):
        sys.path.insert(0, p)
    import concourse.bass as bass  # noqa: F401

import concourse.bacc as bacc
import concourse.tile as tile
from concourse import mybir
from concourse.bass_utils import run_bass_kernel_spmd
from concourse.masks import make_identity

F32 = mybir.dt.float32
BF16 = mybir.dt.bfloat16
FP8 = mybir.dt.float8e4
I16 = mybir.dt.int16
I32 = mybir.dt.int32
AF = mybir.ActivationFunctionType
ALU = mybir.AluOpType
AX = mybir.AxisListType
DR = mybir.MatmulPerfMode.DoubleRow

B, D, C = 1024, 512, 50000
NCORES = 8
CS = C // NCORES          # 6250 real classes per core
NT = 49                   # class tiles of 128 per core
CP = NT * 128             # 6272 padded classes per core
KC = D // 128             # 4 contraction chunks (2 DoubleRow pairs)
G = B // 128              # 8 batch groups of 128
CH = 7                    # rsqrt batching chunk (NT = 7 * 7)
NCH = NT // CH

SCALE = 64.0
MARGIN = 0.5
COS_M = float(np.cos(MARGIN))
SIN_M = float(np.sin(MARGIN))
TH = float(np.cos(np.pi - MARGIN))
MM = float(np.sin(np.pi - MARGIN) * MARGIN)

WS = 256.0                # host fp8 scale on w
ES = 64.0                 # device fp8 scale on normalized e
SC_F = SCALE / (WS * ES)  # exp activation scale factor: 1/256
A16 = 128.0 / float(np.log(2.0))
B16 = 16250.6
ASC_F = A16 * SC_F

DVE_TILE = lambda t: t % 12 == 5      # 4 tiles take the DVE exp path
SCALAR_NORM = lambda t: t % 5 == 2    # 10 norm tiles on ScalarE, 39 on DVE

_CACHED_NC = None


def _newton_rsqrt(nc, pool, s_ap, out_ap, n, iters=1, name="nr"):
    """out = 1/sqrt(s) on DVE via bit-trick seed + slim Newton steps."""
    j = pool.tile([128, n], I32, tag=f"{name}_j", bufs=2, name=f"{name}_j")
    nc.vector.tensor_scalar(j[:], s_ap.bitcast(I32), 1, None,
                            op0=ALU.arith_shift_right)
    nc.vector.tensor_scalar(j[:], j[:], -1, 0x5F3759DF, op0=ALU.mult, op1=ALU.add)
    jb = j[:].bitcast(F32)
    t = pool.tile([128, n], F32, tag=f"{name}_t", bufs=2, name=f"{name}_t")
    r = jb
    for it in range(iters):
        nc.vector.tensor_mul(t[:], r, r)
        nc.vector.tensor_mul(t[:], t[:], s_ap)
        nc.vector.tensor_scalar(t[:], t[:], -0.5, 1.5, op0=ALU.mult, op1=ALU.add)
        nc.vector.tensor_mul(out_ap, r, t[:])
        r = out_ap


def build_kernel():
    nc = bacc.Bacc("TRN2", target_bir_lowering=False, num_devices=NCORES)

    emb_d = nc.declare_dram_parameter("emb", [B, D], F32, isOutput=False)
    wt8_d = nc.declare_dram_parameter("wt8", [128, KC, CP], FP8, isOutput=False)
    wrow_d = nc.declare_dram_parameter("wrow", [CP, D], BF16, isOutput=False)
    lblg_d = nc.declare_dram_parameter("lblg", [128, G], I32, isOutput=False)
    coff_d = nc.declare_dram_parameter("coff", [128, 1], F32, isOutput=False)
    out_d = nc.declare_dram_parameter("out", [1, 1], F32, isOutput=True)

    with tile.TileContext(nc) as tc:
        with (
            tc.tile_pool(name="const", bufs=1) as cpool,
            tc.tile_pool(name="big", bufs=1) as big,
            tc.tile_pool(name="nr", bufs=2) as nrp,
            tc.tile_pool(name="sc", bufs=3) as scp,
            tc.tile_pool(name="ex", bufs=4) as exp_pool,
            tc.tile_pool(name="tgt", bufs=1) as tgt,
            tc.tile_pool(name="pt_ps", bufs=2, space="PSUM") as pt_ps,
            tc.tile_pool(name="cos_ps", bufs=3, space="PSUM") as cos_ps,
            tc.tile_pool(name="dram", bufs=1, space="DRAM") as dpool,
        ):
            # -------- prologue DMAs: embeddings first on the fast ring -----
            e_sb = big.tile([128, G, D], F32)
            for q in range(2):
                nc.gpsimd.dma_start(
                    e_sb[:, q * 4:(q + 1) * 4, :],
                    emb_d[q * 512:(q + 1) * 512, :].rearrange(
                        "(g p) d -> p g d", p=128
                    ),
                )
            coff_sb = cpool.tile([128, 1], F32)
            nc.sync.dma_start(coff_sb[:], coff_d[:, :])
            lblg_sb = cpool.tile([128, G], I32)
            nc.sync.dma_start(lblg_sb[:], lblg_d[:, :])

            wr_sb = big.tile([128, NT, D], BF16)
            tq = (13, 12, 12, 12)
            t0 = 0
            for q in range(4):
                nc.gpsimd.dma_start(
                    wr_sb[:, t0:t0 + tq[q], :],
                    wrow_d[t0 * 128:(t0 + tq[q]) * 128, :].rearrange(
                        "(t p) d -> p t d", p=128
                    ),
                )
                t0 += tq[q]
            wT_sb = big.tile([128, KC, CP], FP8)
            cs_per = CP // 4  # 1568
            for q in range(4):
                nc.sync.dma_start(
                    wT_sb[:, :, q * cs_per:(q + 1) * cs_per],
                    wt8_d[:, :, q * cs_per:(q + 1) * cs_per],
                )

            ident = cpool.tile([128, 128], F32)
            make_identity(nc, ident[:])
            ident_bf = cpool.tile([128, 128], BF16)
            nc.vector.tensor_copy(ident_bf[:], ident[:])

            # kick off the target-row gathers as early as possible
            lf = tgt.tile([128, G], F32)
            nc.vector.tensor_copy(lf[:], lblg_sb[:])          # i32 -> f32
            loc = tgt.tile([128, G], F32)
            nc.vector.tensor_scalar_sub(loc[:], lf[:], coff_sb[:])
            m1 = tgt.tile([128, G], F32)
            nc.vector.tensor_scalar(m1[:], loc[:], 0.0, None, op0=ALU.is_ge)
            m2 = tgt.tile([128, G], F32)
            nc.vector.tensor_scalar(m2[:], loc[:], float(CS), None, op0=ALU.is_lt)
            maskt = tgt.tile([128, G], F32)
            nc.vector.tensor_mul(maskt[:], m1[:], m2[:])
            locc = tgt.tile([128, G], F32)
            nc.vector.tensor_scalar_max(locc[:], loc[:], 0.0)
            nc.vector.tensor_scalar_min(locc[:], locc[:], float(CS - 1))
            loci = tgt.tile([128, G], I32)
            nc.vector.tensor_copy(loci[:], locc[:])            # f32 -> i32
            wt_g = big.tile([128, G, D], BF16)
            for g in range(G):
                nc.gpsimd.indirect_dma_start(
                    out=wt_g[:, g, :],
                    out_offset=None,
                    in_=wrow_d[:, :],
                    in_offset=bass.IndirectOffsetOnAxis(ap=loci[:, g:g + 1], axis=0),
                )

            # ---------------- embedding prep -------------------------------
            es_sq = big.tile([128, G], F32)
            for g in range(G):
                scr = nrp.tile([128, D], F32, tag="escr", bufs=2, name="escr")
                nc.scalar.activation(
                    scr[:], e_sb[:, g], AF.Square, accum_out=es_sq[:, g:g + 1]
                )
            es_r = big.tile([128, G], F32)
            _newton_rsqrt(nc, nrp, es_sq[:], es_r[:], G, iters=2, name="enr")
            es_r64 = big.tile([128, G], F32)
            nc.vector.tensor_scalar_mul(es_r64[:], es_r[:], ES)
            en_bf = big.tile([128, G, D], BF16)
            for g in range(G):
                nc.scalar.activation(
                    en_bf[:, g], e_sb[:, g], AF.Copy, scale=es_r64[:, g:g + 1]
                )
            enT = big.tile([128, KC, B], FP8)
            for g in range(G):
                ptp = pt_ps.tile([128, KC * 128], BF16, tag="ptp", name="ptp", bufs=2)
                for k in range(KC):
                    nc.tensor.transpose(
                        ptp[:, k * 128:(k + 1) * 128],
                        en_bf[:, g, k * 128:(k + 1) * 128],
                        ident_bf[:],
                    )
                nc.scalar.copy(
                    enT[:, :, g * 128:(g + 1) * 128],
                    ptp[:].rearrange("p (k j) -> p k j", k=KC),
                )

            ar_in = dpool.tile([3, 128, G], F32)
            ar_out = dpool.tile([3, 128, G], F32, addr_space="Shared")

            def target_path():
                tssq = tgt.tile([128, G], F32)
                tdot = tgt.tile([128, G], F32)
                for g in range(G):
                    scrb = nrp.tile([128, D], BF16, tag="scrb", bufs=2,
                                    name="scrb")
                    nc.vector.scalar_tensor_tensor(
                        out=scrb[:], in0=wt_g[:, g], scalar=1.0, in1=wt_g[:, g],
                        op0=ALU.mult, op1=ALU.mult,
                        accum_out=tssq[:, g:g + 1],
                    )
                    scrb2 = nrp.tile([128, D], BF16, tag="scrb", bufs=2,
                                     name="scrb2")
                    nc.vector.scalar_tensor_tensor(
                        out=scrb2[:], in0=en_bf[:, g], scalar=1.0,
                        in1=wt_g[:, g],
                        op0=ALU.mult, op1=ALU.mult,
                        accum_out=tdot[:, g:g + 1],
                    )
                trs = tgt.tile([128, G], F32)
                _newton_rsqrt(nc, nrp, tssq[:], trs[:], G, iters=2, name="tnr")
                ct = tgt.tile([128, G], F32)
                nc.vector.tensor_mul(ct[:], tdot[:], trs[:])
                nc.vector.tensor_scalar_mul(ct[:], ct[:], 1.0 / ES)
                t2 = tgt.tile([128, G], F32)
                nc.vector.tensor_mul(t2[:], ct[:], ct[:])
                nc.vector.tensor_scalar_min(t2[:], t2[:], 1.0)
                u = tgt.tile([128, G], F32)
                nc.vector.tensor_scalar(u[:], t2[:], -1.0, 1.0,
                                        op0=ALU.mult, op1=ALU.add)
                nc.vector.tensor_scalar_max(u[:], u[:], 1e-12)
                ur = tgt.tile([128, G], F32)
                _newton_rsqrt(nc, nrp, u[:], ur[:], G, iters=2, name="unr")
                sint = tgt.tile([128, G], F32)
                nc.vector.tensor_mul(sint[:], u[:], ur[:])
                ctcm = tgt.tile([128, G], F32)
                nc.vector.tensor_scalar_mul(ctcm[:], ct[:], COS_M)
                phi = tgt.tile([128, G], F32)
                nc.vector.scalar_tensor_tensor(
                    out=phi[:], in0=sint[:], scalar=-SIN_M, in1=ctcm[:],
                    op0=ALU.mult, op1=ALU.add,
                )
                phif = tgt.tile([128, G], F32)
                nc.vector.tensor_scalar_sub(phif[:], ct[:], MM)
                cmp = tgt.tile([128, G], I32)
                nc.vector.tensor_scalar(cmp[:], ct[:], TH, None, op0=ALU.is_gt)
                nc.vector.copy_predicated(phif[:], cmp[:], phi[:])
                e1 = tgt.tile([128, G], F32)
                nc.scalar.activation(e1[:], phif[:], AF.Exp, scale=SCALE)
                e2 = tgt.tile([128, G], F32)
                nc.scalar.activation(e2[:], ct[:], AF.Exp, scale=SCALE)
                dd = tgt.tile([128, G], F32)
                nc.vector.tensor_sub(dd[:], e1[:], e2[:])
                nc.vector.tensor_mul(dd[:], dd[:], maskt[:])
                pp = tgt.tile([128, G], F32)
                nc.vector.tensor_mul(pp[:], phif[:], maskt[:])
                nc.sync.dma_start(ar_in[1], dd[:])
                nc.sync.dma_start(ar_in[2], pp[:])

            # ---------------- main class-tile loop -------------------------
            ssq_all = big.tile([128, NT], F32)
            acc0 = big.tile([128, B], BF16)
            acc1 = big.tile([128, B], BF16)
            nc.vector.memset(acc0[:], 0.0)
            nc.vector.memset(acc1[:], 0.0)

            def norms(c):
                c0 = c * CH
                for t in range(c0, c0 + CH):
                    if SCALAR_NORM(t):
                        scr = nrp.tile([128, D], F32, tag="escr", bufs=2,
                                       name="nscr")
                        nc.scalar.activation(
                            scr[:], wr_sb[:, t], AF.Square,
                            accum_out=ssq_all[:, t:t + 1],
                        )
                    else:
                        sqb = nrp.tile([128, D], BF16, tag="scrb", bufs=2,
                                       name="sqb")
                        nc.vector.scalar_tensor_tensor(
                            out=sqb[:], in0=wr_sb[:, t], scalar=1.0,
                            in1=wr_sb[:, t],
                            op0=ALU.mult, op1=ALU.mult,
                            accum_out=ssq_all[:, t:t + 1],
                        )

            norms(0)
            for c in range(NCH):
                c0 = c * CH
                rs_c = scp.tile([128, CH], F32, tag="rs_c", bufs=3, name="rs_c")
                _newton_rsqrt(nc, nrp, ssq_all[:, c0:c0 + CH], rs_c[:], CH, iters=1)
                sc_c = scp.tile([128, CH], F32, tag="sc_c", bufs=3, name="sc_c")
                nc.vector.tensor_scalar_mul(sc_c[:], rs_c[:], SC_F)
                asc_c = scp.tile([128, CH], F32, tag="asc_c", bufs=3, name="asc_c")
                nc.vector.tensor_scalar_mul(asc_c[:], rs_c[:], ASC_F)
                if c + 1 < NCH:
                    norms(c + 1)
                if c == 2:
                    target_path()
                for t in range(c0, c0 + CH):
                    i = t - c0
                    cps = cos_ps.tile([128, B], F32, tag="cos", name="cps")
                    for kp in range(2):
                        for h in range(2):
                            nc.tensor.matmul(
                                cps[:, h * 512:(h + 1) * 512],
                                lhsT=wT_sb[:, 2 * kp:2 * kp + 2,
                                           t * 128:(t + 1) * 128],
                                rhs=enT[:, 2 * kp:2 * kp + 2,
                                        h * 512:(h + 1) * 512],
                                start=(kp == 0),
                                stop=(kp == 1),
                                perf_mode=DR,
                            )
                    a = acc0 if t % 2 == 0 else acc1
                    if DVE_TILE(t):
                        it = exp_pool.tile([128, B], I16, tag="it", name="it",
                                           bufs=2)
                        nc.vector.tensor_scalar(
                            it[:], cps[:], asc_c[:, i:i + 1], B16,
                            op0=ALU.mult, op1=ALU.add,
                        )
                        nc.vector.tensor_add(a[:], a[:], it[:].bitcast(BF16))
                    else:
                        ex = exp_pool.tile([128, B], BF16, tag="ex", name="ex",
                                           bufs=3)
                        nc.scalar.activation(
                            ex[:], cps[:], AF.Exp, scale=sc_c[:, i:i + 1]
                        )
                        nc.vector.tensor_add(a[:], a[:], ex[:])

            # ---------------- reduce partials over class partitions ---------
            nc.vector.tensor_add(acc0[:], acc0[:], acc1[:])
            ptr = pt_ps.tile([128, G, 128], BF16, tag="ptp", name="ptr", bufs=2)
            for g in range(G):
                nc.tensor.transpose(
                    ptr[:, g, :], acc0[:, g * 128:(g + 1) * 128], ident_bf[:]
                )
            S_sb = tgt.tile([128, G], F32)
            nc.vector.reduce_sum(S_sb[:], ptr[:], axis=AX.X)
            nc.sync.dma_start(ar_in[0], S_sb[:])

            # ---------------- AllReduce + finale ----------------
            nc.gpsimd.collective_compute(
                "AllReduce",
                ALU.add,
                replica_groups=[list(range(NCORES))],
                ins=[ar_in[:].opt()],
                outs=[ar_out[:].opt()],
            )
            Sg = tgt.tile([128, G], F32)
            Dg = tgt.tile([128, G], F32)
            Pg = tgt.tile([128, G], F32)
            nc.sync.dma_start(Sg[:], ar_out[0])
            nc.sync.dma_start(Dg[:], ar_out[1])
            nc.sync.dma_start(Pg[:], ar_out[2])

            Zt = tgt.tile([128, G], F32)
            nc.vector.tensor_add(Zt[:], Sg[:], Dg[:])
            lnz = tgt.tile([128, G], F32)
            nc.scalar.activation(lnz[:], Zt[:], AF.Ln)
            nll = tgt.tile([128, G], F32)
            nc.vector.scalar_tensor_tensor(
                out=nll[:], in0=Pg[:], scalar=-SCALE, in1=lnz[:],
                op0=ALU.mult, op1=ALU.add,
            )
            csum = tgt.tile([128, 1], F32)
            nc.vector.reduce_sum(csum[:], nll[:], axis=AX.X)
            ptf = pt_ps.tile([128, 512], F32, tag="ptp", name="ptf", bufs=2)
            nc.tensor.transpose(ptf[:1, 0:128], csum[:], ident[:])
            fin = tgt.tile([1, 1], F32)
            nc.vector.tensor_reduce(fin[:], ptf[:1, 0:128], axis=AX.X, op=ALU.add)
            nc.vector.tensor_scalar_mul(fin[:], fin[:], 1.0 / B)
            nc.sync.dma_start(out_d[:, :], fin[:])

    nc.compile()
    return nc


def _shard_inputs(embeddings, labels, weight):
    emb = np.ascontiguousarray(embeddings, dtype=np.float32)
    lbl = np.ascontiguousarray(labels, dtype=np.int32)
    w = np.ascontiguousarray(weight, dtype=np.float32)
    lblg = np.ascontiguousarray(lbl.reshape(G, 128).T)
    in_maps = []
    for i in range(NCORES):
        wsh = np.empty((CP, D), np.float32)
        wsh[:CS] = w[i * CS:(i + 1) * CS]
        wsh[CS:] = 1.0  # pad rows: finite norms; fp8 image has them zeroed
        wrow = wsh.astype(ml_dtypes.bfloat16)
        w8 = np.zeros((CP, D), np.float32)
        w8[:CS] = wsh[:CS] * WS
        w8 = w8.astype(ml_dtypes.float8_e4m3)       # [CP, D]
        wt8 = np.ascontiguousarray(
            w8.T.reshape(KC, 128, CP).transpose(1, 0, 2)
        )
        coff = np.full((128, 1), i * CS, np.float32)
        in_maps.append(
            {"emb": emb, "wt8": wt8, "wrow": wrow, "lblg": lblg, "coff": coff}
        )
    return in_maps


def kernel(embeddings, labels, weight):
    global _CACHED_NC
    if _CACHED_NC is None:
        _CACHED_NC = build_kernel()
    in_maps = _shard_inputs(embeddings, labels, weight)
    res = run_bass_kernel_spmd(_CACHED_NC, in_maps, core_ids=list(range(NCORES)))
    return np.float32(res.results[0]["out"][0, 0])


if __name__ == "__main__":
    rng = np.random.default_rng(0)
    emb = rng.standard_normal((B, D), dtype=np.float32)
    lbl = rng.integers(0, C, size=(B,), dtype=np.int32)
    w = (rng.random((C, D), dtype=np.float32) - 0.5) * 0.02
    print("loss =", kernel(emb, lbl, w))
